# revision 33
# baseline (speedup 1.0000x reference)
"""Trainium2 Bass kernel for nn_BinLoss (SmoothL1 + histogram-diff loss).

Contract: kernel(**inputs) takes FULL inputs
    inp: [8, 11, 64, 64, 64] f32
    tar: [8, 11, 64, 64, 64] f32
    bin_range: [20, 2] f32
and returns the full output (f32 scalar), matching

    loss1 = SmoothL1(inp, tar)          (beta=1, mean)
    h(x)[b,c,k] = count(x[b,c] in [lo_k, hi_k)) / nvox
    loss2 = mean |h(inp) - h(tar)|
    out  = 0.5*loss1 + 0.5*loss2

Strategy: data-parallel over batch (8 cores, 1 batch element each); no
collectives — each core owns complete per-(b,c) histograms and partial
SmoothL1 sums, the host combines ~KB of stats in float64.

Per-core pipeline (three engine lanes, per channel):
  - DVE lane: cast x,y -> bf16; per "PE edge" generate a 0/1 mask via
    tensor_scalar(is_ge) in bf16 4x mode (no accum_out - accum forces
    1x mode, measured); SmoothL1 d=x-y and min(|d|,1) with accum.
  - TensorE lane: reduce each mask with one-hot-column lhsT matmuls
    into a per-channel PSUM [units, 512] accumulator (row r collects
    edge r's partition-sums; PSUM accumulates across the 4 column
    chunks and all units). One DVE tensor_reduce evacuates it.
  - ACT lane: the 5 most-central edges per tensor are counted exactly
    in f32 via Sign(x - e) with fused accumulation (count_ge =
    (N + sum sign)/2), plus |d| and min(|d|,1)^2 accumulation for
    SmoothL1 (identity: smoothl1 = 0.5 m^2 + |d| - m, m=min(|d|,1)).
"""

from contextlib import ExitStack

import numpy as np

import concourse.bacc as bacc
import concourse.bass as bass
import concourse.mybir as mybir
import concourse.tile as tile
from concourse.bass_utils import run_bass_kernel_spmd

N_CORES = 8
B, C = 8, 11
NVOX = 64 * 64 * 64  # 262144
P = 128
F = NVOX // P  # 2048
NCHUNK = 4  # F/512 matmul chunks
N_ACT = 5   # most-central edges per tensor counted on ACT (exact f32)

f32 = mybir.dt.float32
bf16 = mybir.dt.bfloat16
AF = mybir.ActivationFunctionType
ALU = mybir.AluOpType


def _n_act(ne):
    return min(N_ACT, ne)


def _split_edges(edges):
    """Return (act_edges, pe_edges): the most-central edges on ACT."""
    order = sorted(range(len(edges)), key=lambda i: abs(edges[i]))
    act = sorted(order[:_n_act(len(edges))])
    pe = sorted(set(range(len(edges))) - set(act))
    return act, pe


def _build_program(edges: list[float]):
    ne = len(edges)
    na = _n_act(ne)
    act_idx, pe_idx = _split_edges(edges)
    n_pe = len(pe_idx)          # PE-lane edges per tensor
    units = 2 * n_pe            # PSUM rows (x edges then y edges)
    assert units <= 128
    ue = max(units, 1)          # avoid zero-size eye tensor

    # stats layouts
    #   dve: [m(c)]                                  -> C cols
    #   act: [u(c) | q(c) | sx(c,a) | sy(c,a)]       -> 2C + 2*C*na
    #   pe : [units rows x C cols]
    ncol_dve = C
    ncol_act = 2 * C + 2 * C * max(na, 1)

    nc = bacc.Bacc("TRN2", target_bir_lowering=False, debug=False,
                   num_devices=N_CORES)
    inp_d = nc.dram_tensor("inp", [C, P, F], f32, kind="ExternalInput").ap()
    tar_d = nc.dram_tensor("tar", [C, P, F], f32, kind="ExternalInput").ap()
    eye_d = nc.dram_tensor("eye", [P, ue * ue], bf16,
                           kind="ExternalInput").ap()
    abias_d = nc.dram_tensor("abias", [P, max(na, 1)], f32,
                             kind="ExternalInput").ap()
    sd_d = nc.dram_tensor("stats_dve", [P, ncol_dve], f32,
                          kind="ExternalOutput").ap()
    sa_d = nc.dram_tensor("stats_act", [P, ncol_act], f32,
                          kind="ExternalOutput").ap()
    sp_d = nc.dram_tensor("stats_pe", [P, C], f32,
                          kind="ExternalOutput").ap()

    with tile.TileContext(nc) as tc, ExitStack() as ctx:
        io_pool = ctx.enter_context(tc.tile_pool(name="io", bufs=3))
        bfp = ctx.enter_context(tc.tile_pool(name="bfp", bufs=2))
        wk_pool = ctx.enter_context(tc.tile_pool(name="wk", bufs=2))
        mk_pool = ctx.enter_context(tc.tile_pool(name="mk", bufs=14))
        st_pool = ctx.enter_context(tc.tile_pool(name="st", bufs=1))
        ps_pool = ctx.enter_context(
            tc.tile_pool(name="ps", bufs=4, space="PSUM"))

        sd = st_pool.tile([P, ncol_dve], f32, tag="sd")
        sa = st_pool.tile([P, ncol_act], f32, tag="sa")
        sp = st_pool.tile([P, C], f32, tag="sp")
        nc.vector.memset(sp[:], 0.0)
        eye = st_pool.tile([P, ue * ue], bf16, tag="eye")
        nc.gpsimd.dma_start(eye[:], eye_d[:])
        abias = st_pool.tile([P, max(na, 1)], f32, tag="abias")
        nc.gpsimd.dma_start(abias[:], abias_d[:])

        nab = max(na, 1)

        def col_a(q, c, a=0):
            base = {"u": 0, "q": C, "sx": 2 * C, "sy": 2 * C + C * nab}[q]
            idx = base + (c * nab + a if q in ("sx", "sy") else c)
            return sa[:, idx:idx + 1]

        for c in range(C):
            x = io_pool.tile([P, F], f32, tag="x")
            nc.sync.dma_start(x[:], inp_d[c])
            y = io_pool.tile([P, F], f32, tag="y")
            nc.sync.dma_start(y[:], tar_d[c])
            xb = bfp.tile([P, F], bf16, tag="xb")
            nc.vector.tensor_copy(xb[:], x[:])
            yb = bfp.tile([P, F], bf16, tag="yb")
            nc.vector.tensor_copy(yb[:], y[:])

            # ---- ACT lane first: sign ops only need x/y, keep the
            # in-order ACT queue from stalling on the late d tile ----
            sgn = wk_pool.tile([P, F], bf16, tag="sgn")
            for a, j in enumerate(act_idx):
                nc.scalar.activation(sgn[:], x[:], AF.Sign,
                                     bias=abias[:, a:a + 1],
                                     accum_out=col_a("sx", c, a))
                nc.scalar.activation(sgn[:], y[:], AF.Sign,
                                     bias=abias[:, a:a + 1],
                                     accum_out=col_a("sy", c, a))

            # ---- PE lane: bf16 masks + one-hot matmul reduction ----
            if units:
                ps = ps_pool.tile([units, 512], f32, tag="ps")
                first = True
                for r, (src, j) in enumerate(
                        [("x", j) for j in pe_idx]
                        + [("y", j) for j in pe_idx]):
                    mask = mk_pool.tile([P, F], bf16, tag="mask")
                    nc.vector.tensor_scalar(
                        out=mask[:], in0=(xb if src == "x" else yb)[:],
                        scalar1=float(edges[j]), scalar2=None, op0=ALU.is_ge)
                    lhs = eye[:, r * units:(r + 1) * units]
                    for k in range(NCHUNK):
                        nc.tensor.matmul(
                            ps[:], lhs, mask[:, k * 512:(k + 1) * 512],
                            start=first,
                            stop=(r == units - 1 and k == NCHUNK - 1))
                        first = False

                nc.vector.tensor_reduce(out=sp[0:units, c:c + 1], in_=ps[:],
                                        op=ALU.add, axis=mybir.AxisListType.X)

            # ---- SmoothL1 partials, after masks so PE starts early
            # (d in bf16: 2x TT mode; |d| error ~0.4% random per
            # element, averages out over 23M) ----
            d = wk_pool.tile([P, F], bf16, tag="d")
            nc.vector.tensor_tensor(out=d[:], in0=xb[:], in1=yb[:],
                                    op=ALU.subtract)
            u = wk_pool.tile([P, F], f32, tag="u")
            nc.scalar.activation(u[:], d[:], AF.Abs, accum_out=col_a("u", c))
            m = wk_pool.tile([P, F], f32, tag="m")
            nc.vector.tensor_scalar(out=m[:], in0=u[:], scalar1=1.0,
                                    scalar2=None, op0=ALU.min, op1=ALU.add,
                                    accum_out=sd[:, c:c + 1])
            q = wk_pool.tile([P, F], f32, tag="u")
            nc.scalar.activation(q[:], m[:], AF.Square, accum_out=col_a("q", c))

        nc.gpsimd.dma_start(sd_d[:, :], sd[:])
        nc.gpsimd.dma_start(sa_d[:, :], sa[:])
        nc.gpsimd.dma_start(sp_d[:, :C], sp[:, :])
    nc.compile()
    return nc


_PROG_CACHE: dict = {}


def _get_program(edges_key):
    if edges_key not in _PROG_CACHE:
        _PROG_CACHE[edges_key] = _build_program(list(edges_key))
    return _PROG_CACHE[edges_key]


def kernel(inp: np.ndarray, tar: np.ndarray, bin_range: np.ndarray,
           _run=None) -> np.ndarray:
    import ml_dtypes

    inp = np.ascontiguousarray(inp, dtype=np.float32)
    tar = np.ascontiguousarray(tar, dtype=np.float32)
    br = np.asarray(bin_range, dtype=np.float32)

    edges = []
    for v in br.reshape(-1):
        fv = float(v)
        if fv not in edges:
            edges.append(fv)
    ne = len(edges)
    na = _n_act(ne)
    nab = max(na, 1)
    eidx = {e: i for i, e in enumerate(edges)}
    act_idx, pe_idx = _split_edges(edges)
    n_pe = len(pe_idx)
    units = 2 * n_pe
    ue = max(units, 1)

    nc = _get_program(tuple(edges))

    eye = np.zeros((P, ue, ue), dtype=ml_dtypes.bfloat16)
    for r in range(units):
        eye[:, r, r] = 1
    eye = eye.reshape(P, ue * ue)
    abias = np.zeros((P, nab), np.float32)
    if na:
        abias[:] = -np.float32([edges[j] for j in act_idx]).reshape(1, na)

    in_maps = []
    for b in range(B):
        in_maps.append({
            "inp": inp[b].reshape(C, P, F),
            "tar": tar[b].reshape(C, P, F),
            "eye": eye,
            "abias": abias.astype(np.float32),
        })
    runner = _run if _run is not None else run_bass_kernel_spmd
    res = runner(nc, in_maps, list(range(N_CORES)))
    results = res.results if hasattr(res, "results") else res

    # ---- host-side tiny combine (float64) ----
    sum_u = sum_m = sum_q = 0.0
    # cge[b, tensor, c, edge]
    cge = np.zeros((B, 2, C, ne), np.float64)
    for b in range(B):
        sd = results[b]["stats_dve"].astype(np.float64)
        sa = results[b]["stats_act"].astype(np.float64)
        sp = results[b]["stats_pe"].astype(np.float64)
        sum_m += sd[:, 0:C].sum()
        sum_u += sa[:, 0:C].sum()
        sum_q += sa[:, C:2 * C].sum()
        # ACT lane: count_ge = (NVOX + sum sign)/2
        for a, j in enumerate(act_idx):
            sx = sa[:, 2 * C + np.arange(C) * nab + a].sum(axis=0)
            sy = sa[:, 2 * C + C * nab + np.arange(C) * nab + a].sum(axis=0)
            cge[b, 0, :, j] = (NVOX + sx) / 2.0
            cge[b, 1, :, j] = (NVOX + sy) / 2.0
        # PE lane: stats_pe[r, c] is the full count for unit r
        for r, j in enumerate(pe_idx):
            cge[b, 0, :, j] = sp[r, :C]
            cge[b, 1, :, j] = sp[n_pe + r, :C]

    hist_i = np.zeros((B, C, br.shape[0]), np.float64)
    hist_t = np.zeros((B, C, br.shape[0]), np.float64)
    for k in range(br.shape[0]):
        lo, hi = float(br[k, 0]), float(br[k, 1])
        if lo < hi:
            hist_i[:, :, k] = cge[:, 0, :, eidx[lo]] - cge[:, 0, :, eidx[hi]]
            hist_t[:, :, k] = cge[:, 1, :, eidx[lo]] - cge[:, 1, :, eidx[hi]]
    hist_i /= NVOX
    hist_t /= NVOX

    n_el = B * C * NVOX
    loss1 = (0.5 * sum_q + sum_u - sum_m) / n_el
    loss2 = np.abs(hist_i - hist_t).mean()
    return np.float32(0.5 * loss1 + 0.5 * loss2)


# revision 34
# speedup vs baseline: 1.0075x; 1.0075x over previous
"""Trainium2 Bass kernel for nn_BinLoss (SmoothL1 + histogram-diff loss).

Contract: kernel(**inputs) takes FULL inputs
    inp: [8, 11, 64, 64, 64] f32
    tar: [8, 11, 64, 64, 64] f32
    bin_range: [20, 2] f32
and returns the full output (f32 scalar), matching

    loss1 = SmoothL1(inp, tar)          (beta=1, mean)
    h(x)[b,c,k] = count(x[b,c] in [lo_k, hi_k)) / nvox
    loss2 = mean |h(inp) - h(tar)|
    out  = 0.5*loss1 + 0.5*loss2

Strategy: data-parallel over batch (8 cores, 1 batch element each); no
collectives — each core owns complete per-(b,c) histograms and partial
SmoothL1 sums, the host combines ~KB of stats in float64.

Per-core pipeline (three engine lanes, per channel):
  - DVE lane: cast x,y -> bf16; per "PE edge" generate a 0/1 mask via
    tensor_scalar(is_ge) in bf16 4x mode (no accum_out - accum forces
    1x mode, measured); SmoothL1 d=x-y and min(|d|,1) with accum.
  - TensorE lane: reduce each mask with one-hot-column lhsT matmuls
    into a per-channel PSUM [units, 512] accumulator (row r collects
    edge r's partition-sums; PSUM accumulates across the 4 column
    chunks and all units). One DVE tensor_reduce evacuates it.
  - ACT lane: the 5 most-central edges per tensor are counted exactly
    in f32 via Sign(x - e) with fused accumulation (count_ge =
    (N + sum sign)/2), plus |d| and min(|d|,1)^2 accumulation for
    SmoothL1 (identity: smoothl1 = 0.5 m^2 + |d| - m, m=min(|d|,1)).
"""

from contextlib import ExitStack

import numpy as np

import concourse.bacc as bacc
import concourse.bass as bass
import concourse.mybir as mybir
import concourse.tile as tile
from concourse.bass_utils import run_bass_kernel_spmd

N_CORES = 8
B, C = 8, 11
NVOX = 64 * 64 * 64  # 262144
P = 128
F = NVOX // P  # 2048
NCHUNK = 4  # F/512 matmul chunks
N_ACT = 5   # most-central edges per tensor counted on ACT (exact f32)

f32 = mybir.dt.float32
bf16 = mybir.dt.bfloat16
AF = mybir.ActivationFunctionType
ALU = mybir.AluOpType


def _n_act(ne):
    return min(N_ACT, ne)


def _split_edges(edges):
    """Return (act_edges, pe_edges): the most-central edges on ACT."""
    order = sorted(range(len(edges)), key=lambda i: abs(edges[i]))
    act = sorted(order[:_n_act(len(edges))])
    pe = sorted(set(range(len(edges))) - set(act))
    return act, pe


def _build_program(edges: list[float]):
    ne = len(edges)
    na = _n_act(ne)
    act_idx, pe_idx = _split_edges(edges)
    n_pe = len(pe_idx)          # PE-lane edges per tensor
    units = 2 * n_pe            # PSUM rows (x edges then y edges)
    assert units <= 128
    ue = max(units, 1)          # avoid zero-size eye tensor

    # stats layouts
    #   dve: [m(c)]                                  -> C cols
    #   act: [u(c) | q(c) | sx(c,a) | sy(c,a)]       -> 2C + 2*C*na
    #   pe : [units rows x C cols]
    ncol_dve = C
    ncol_act = 2 * C + 2 * C * max(na, 1)

    nc = bacc.Bacc("TRN2", target_bir_lowering=False, debug=False,
                   num_devices=N_CORES)
    inp_d = nc.dram_tensor("inp", [C, P, F], f32, kind="ExternalInput").ap()
    tar_d = nc.dram_tensor("tar", [C, P, F], f32, kind="ExternalInput").ap()
    eye_d = nc.dram_tensor("eye", [P, ue * ue], bf16,
                           kind="ExternalInput").ap()
    abias_d = nc.dram_tensor("abias", [P, max(na, 1)], f32,
                             kind="ExternalInput").ap()
    sd_d = nc.dram_tensor("stats_dve", [P, ncol_dve], f32,
                          kind="ExternalOutput").ap()
    sa_d = nc.dram_tensor("stats_act", [P, ncol_act], f32,
                          kind="ExternalOutput").ap()
    sp_d = nc.dram_tensor("stats_pe", [P, C], f32,
                          kind="ExternalOutput").ap()

    with tile.TileContext(nc) as tc, ExitStack() as ctx:
        io_pool = ctx.enter_context(tc.tile_pool(name="io", bufs=3))
        bfp = ctx.enter_context(tc.tile_pool(name="bfp", bufs=2))
        wk_pool = ctx.enter_context(tc.tile_pool(name="wk", bufs=2))
        mk_pool = ctx.enter_context(tc.tile_pool(name="mk", bufs=12))
        st_pool = ctx.enter_context(tc.tile_pool(name="st", bufs=1))
        ps_pool = ctx.enter_context(
            tc.tile_pool(name="ps", bufs=3, space="PSUM"))

        sd = st_pool.tile([P, ncol_dve], f32, tag="sd")
        sa = st_pool.tile([P, ncol_act], f32, tag="sa")
        sp = st_pool.tile([P, C], f32, tag="sp")
        nc.vector.memset(sp[:], 0.0)
        eye = st_pool.tile([P, ue * ue], bf16, tag="eye")
        nc.gpsimd.dma_start(eye[:], eye_d[:])
        abias = st_pool.tile([P, max(na, 1)], f32, tag="abias")
        nc.gpsimd.dma_start(abias[:], abias_d[:])

        nab = max(na, 1)

        def col_a(q, c, a=0):
            base = {"u": 0, "q": C, "sx": 2 * C, "sy": 2 * C + C * nab}[q]
            idx = base + (c * nab + a if q in ("sx", "sy") else c)
            return sa[:, idx:idx + 1]

        for c in range(C):
            x = io_pool.tile([P, F], f32, tag="x")
            nc.sync.dma_start(x[:], inp_d[c])
            y = io_pool.tile([P, F], f32, tag="y")
            nc.sync.dma_start(y[:], tar_d[c])
            xb = bfp.tile([P, F], bf16, tag="xb")
            nc.vector.tensor_copy(xb[:], x[:])
            yb = bfp.tile([P, F], bf16, tag="yb")
            nc.vector.tensor_copy(yb[:], y[:])

            # ---- ACT lane first: sign ops only need x/y, keep the
            # in-order ACT queue from stalling on the late d tile ----
            sgn = wk_pool.tile([P, F], bf16, tag="sgn")
            for a, j in enumerate(act_idx):
                nc.scalar.activation(sgn[:], x[:], AF.Sign,
                                     bias=abias[:, a:a + 1],
                                     accum_out=col_a("sx", c, a))
                nc.scalar.activation(sgn[:], y[:], AF.Sign,
                                     bias=abias[:, a:a + 1],
                                     accum_out=col_a("sy", c, a))

            # ---- PE lane: bf16 masks + one-hot matmul reduction ----
            if units:
                ps = ps_pool.tile([units, 512], f32, tag="ps")
                first = True
                for r, (src, j) in enumerate(
                        [("x", j) for j in pe_idx]
                        + [("y", j) for j in pe_idx]):
                    mask = mk_pool.tile([P, F], bf16, tag="mask")
                    nc.vector.tensor_scalar(
                        out=mask[:], in0=(xb if src == "x" else yb)[:],
                        scalar1=float(edges[j]), scalar2=None, op0=ALU.is_ge)
                    lhs = eye[:, r * units:(r + 1) * units]
                    for k in range(NCHUNK):
                        nc.tensor.matmul(
                            ps[:], lhs, mask[:, k * 512:(k + 1) * 512],
                            start=first,
                            stop=(r == units - 1 and k == NCHUNK - 1))
                        first = False

                nc.vector.tensor_reduce(out=sp[0:units, c:c + 1], in_=ps[:],
                                        op=ALU.add, axis=mybir.AxisListType.X)

            # ---- SmoothL1 partials, after masks so PE starts early
            # (d in bf16: 2x TT mode; |d| error ~0.4% random per
            # element, averages out over 23M) ----
            d = wk_pool.tile([P, F], bf16, tag="d")
            nc.vector.tensor_tensor(out=d[:], in0=xb[:], in1=yb[:],
                                    op=ALU.subtract)
            u = wk_pool.tile([P, F], f32, tag="u")
            nc.scalar.activation(u[:], d[:], AF.Abs, accum_out=col_a("u", c))
            m = wk_pool.tile([P, F], f32, tag="m")
            nc.vector.tensor_scalar(out=m[:], in0=u[:], scalar1=1.0,
                                    scalar2=None, op0=ALU.min, op1=ALU.add,
                                    accum_out=sd[:, c:c + 1])
            q = wk_pool.tile([P, F], f32, tag="u")
            nc.scalar.activation(q[:], m[:], AF.Square, accum_out=col_a("q", c))

        nc.gpsimd.dma_start(sd_d[:, :], sd[:])
        nc.gpsimd.dma_start(sa_d[:, :], sa[:])
        nc.gpsimd.dma_start(sp_d[:, :C], sp[:, :])
    nc.compile()
    return nc


_PROG_CACHE: dict = {}


def _get_program(edges_key):
    if edges_key not in _PROG_CACHE:
        _PROG_CACHE[edges_key] = _build_program(list(edges_key))
    return _PROG_CACHE[edges_key]


def kernel(inp: np.ndarray, tar: np.ndarray, bin_range: np.ndarray,
           _run=None) -> np.ndarray:
    import ml_dtypes

    inp = np.ascontiguousarray(inp, dtype=np.float32)
    tar = np.ascontiguousarray(tar, dtype=np.float32)
    br = np.asarray(bin_range, dtype=np.float32)

    edges = []
    for v in br.reshape(-1):
        fv = float(v)
        if fv not in edges:
            edges.append(fv)
    ne = len(edges)
    na = _n_act(ne)
    nab = max(na, 1)
    eidx = {e: i for i, e in enumerate(edges)}
    act_idx, pe_idx = _split_edges(edges)
    n_pe = len(pe_idx)
    units = 2 * n_pe
    ue = max(units, 1)

    nc = _get_program(tuple(edges))

    eye = np.zeros((P, ue, ue), dtype=ml_dtypes.bfloat16)
    for r in range(units):
        eye[:, r, r] = 1
    eye = eye.reshape(P, ue * ue)
    abias = np.zeros((P, nab), np.float32)
    if na:
        abias[:] = -np.float32([edges[j] for j in act_idx]).reshape(1, na)

    in_maps = []
    for b in range(B):
        in_maps.append({
            "inp": inp[b].reshape(C, P, F),
            "tar": tar[b].reshape(C, P, F),
            "eye": eye,
            "abias": abias.astype(np.float32),
        })
    runner = _run if _run is not None else run_bass_kernel_spmd
    res = runner(nc, in_maps, list(range(N_CORES)))
    results = res.results if hasattr(res, "results") else res

    # ---- host-side tiny combine (float64) ----
    sum_u = sum_m = sum_q = 0.0
    # cge[b, tensor, c, edge]
    cge = np.zeros((B, 2, C, ne), np.float64)
    for b in range(B):
        sd = results[b]["stats_dve"].astype(np.float64)
        sa = results[b]["stats_act"].astype(np.float64)
        sp = results[b]["stats_pe"].astype(np.float64)
        sum_m += sd[:, 0:C].sum()
        sum_u += sa[:, 0:C].sum()
        sum_q += sa[:, C:2 * C].sum()
        # ACT lane: count_ge = (NVOX + sum sign)/2
        for a, j in enumerate(act_idx):
            sx = sa[:, 2 * C + np.arange(C) * nab + a].sum(axis=0)
            sy = sa[:, 2 * C + C * nab + np.arange(C) * nab + a].sum(axis=0)
            cge[b, 0, :, j] = (NVOX + sx) / 2.0
            cge[b, 1, :, j] = (NVOX + sy) / 2.0
        # PE lane: stats_pe[r, c] is the full count for unit r
        for r, j in enumerate(pe_idx):
            cge[b, 0, :, j] = sp[r, :C]
            cge[b, 1, :, j] = sp[n_pe + r, :C]

    hist_i = np.zeros((B, C, br.shape[0]), np.float64)
    hist_t = np.zeros((B, C, br.shape[0]), np.float64)
    for k in range(br.shape[0]):
        lo, hi = float(br[k, 0]), float(br[k, 1])
        if lo < hi:
            hist_i[:, :, k] = cge[:, 0, :, eidx[lo]] - cge[:, 0, :, eidx[hi]]
            hist_t[:, :, k] = cge[:, 1, :, eidx[lo]] - cge[:, 1, :, eidx[hi]]
    hist_i /= NVOX
    hist_t /= NVOX

    n_el = B * C * NVOX
    loss1 = (0.5 * sum_q + sum_u - sum_m) / n_el
    loss2 = np.abs(hist_i - hist_t).mean()
    return np.float32(0.5 * loss1 + 0.5 * loss2)
